# revision 9
# baseline (speedup 1.0000x reference)
"""Mamba mixer (nn_Mixer) Trainium2 Bass kernel.

Sharding: tensor-parallel over d_inner (2048 -> 256 per core, 8 cores).
Each core computes its d_inner shard of: in_proj (x,z halves), causal
conv1d, silu, x_proj partial (all-reduced across cores), dt_proj,
softplus, the selective scan (hardware tensor_tensor_scan along time),
gating, and an out_proj partial.  The 8 out_proj partials are summed on
the host (that sum is the unshard step of row-parallel out_proj), along
with the D_skip*u residual.

Self-contained: hardcodes all shapes; only needs the concourse/bass
runtime that ships in the container.
"""

import os
import numpy as np

# Problem sizes (fixed by the problem statement)
D_MODEL = 1024
D_INNER = 2048
NSTATE = 16
DT_RANK = 64
DCONV = 4
BATCH = 2
SEQ = 4096

NCORES = 8
DS = D_INNER // NCORES          # 256 d_inner rows per core
DT2 = DS // 128                 # 2 partition tiles per core


def _build_nc(seq_len=SEQ, lc=512, fake_collective=False):
    """Build the Bass program (same SPMD program for all 8 cores).

    fake_collective=True replaces the AllReduce with a local DRAM copy so
    the program is single-core simulable (TimelineSim perf estimates).
    """
    import concourse.bass as bass
    import concourse.bacc as bacc
    import concourse.mybir as mybir
    import concourse.tile as tile

    f32 = mybir.dt.float32
    bf16 = mybir.dt.bfloat16
    AF = mybir.ActivationFunctionType
    OP = mybir.AluOpType

    tok = BATCH * seq_len
    units_per_b = seq_len // lc

    nc = bacc.Bacc("TRN2", target_bir_lowering=False, debug=False,
                   num_devices=NCORES)

    # ---- kernel I/O (per-core shards prepared on the host) ----
    uT = nc.dram_tensor("uT", [D_MODEL, tok], bf16, kind="ExternalInput")
    w_in = nc.dram_tensor("w_inT", [D_MODEL, 4 * 128], bf16, kind="ExternalInput")
    conv_w = nc.dram_tensor("conv_w", [128, DT2 * DCONV], f32, kind="ExternalInput")
    conv_b = nc.dram_tensor("conv_b", [128, DT2], f32, kind="ExternalInput")
    w_xp = nc.dram_tensor("w_xpT", [DS, DT_RANK + 2 * NSTATE], bf16, kind="ExternalInput")
    w_dt = nc.dram_tensor("w_dtT", [DT_RANK, DS], bf16, kind="ExternalInput")
    dt_bias = nc.dram_tensor("dt_bias", [128, DT2], f32, kind="ExternalInput")
    a_neg = nc.dram_tensor("a_neg", [128, DT2 * NSTATE], f32, kind="ExternalInput")
    d_in = nc.dram_tensor("d_in", [128, DT2], f32, kind="ExternalInput")
    w_out = nc.dram_tensor("w_outT", [DS, D_MODEL], bf16, kind="ExternalInput")
    eye_d = nc.dram_tensor("eye128", [128, 128], f32, kind="ExternalInput")
    sel_d = nc.dram_tensor("sel32", [2 * NSTATE, 2 * NSTATE * 128], bf16,
                           kind="ExternalInput")
    y_part = nc.dram_tensor("y_part", [D_MODEL, tok], bf16, kind="ExternalOutput")

    NXD = DT_RANK + 2 * NSTATE  # 96

    with tile.TileContext(nc) as tc:
        with (
            tc.tile_pool(name="const", bufs=1) as cpool,
            tc.tile_pool(name="u", bufs=2) as upool,
            tc.tile_pool(name="work", bufs=2) as wpool,
            tc.tile_pool(name="nwork", bufs=3) as npool,
            tc.tile_pool(name="small", bufs=2) as spool,
            tc.tile_pool(name="obuf", bufs=4) as opool,
            tc.tile_pool(name="mm", bufs=2, space="PSUM") as psmm,
            tc.tile_pool(name="psy", bufs=2, space="PSUM") as psy,
            tc.tile_pool(name="psx", bufs=1, space="PSUM") as psx,
            tc.tile_pool(name="psb", bufs=1, space="PSUM") as psb,
            tc.tile_pool(name="dram", bufs=2, space="DRAM") as dpool,
        ):
            # ---- static weights into SBUF ----
            w_in_sb = cpool.tile([128, 8, 4 * 128], bf16)
            nc.sync.dma_start(w_in_sb[:], uT_re := w_in.ap().rearrange(
                "(j p) m -> p j m", p=128))
            w_out_sb = cpool.tile([128, DT2, D_MODEL], bf16)
            nc.sync.dma_start(w_out_sb[:], w_out.ap().rearrange(
                "(k p) m -> p k m", p=128))
            w_xp_sb = cpool.tile([128, DT2, NXD], bf16)
            nc.sync.dma_start(w_xp_sb[:], w_xp.ap().rearrange(
                "(k p) m -> p k m", p=128))
            w_dt_sb = cpool.tile([DT_RANK, DS], bf16)
            nc.sync.dma_start(w_dt_sb[:], w_dt.ap())
            conv_w_sb = cpool.tile([128, DT2 * DCONV], f32)
            nc.sync.dma_start(conv_w_sb[:], conv_w.ap())
            conv_b_sb = cpool.tile([128, DT2], f32)
            nc.sync.dma_start(conv_b_sb[:], conv_b.ap())
            dt_bias_sb = cpool.tile([128, DT2], f32)
            nc.sync.dma_start(dt_bias_sb[:], dt_bias.ap())
            a_sb = cpool.tile([128, DT2 * NSTATE], f32)
            nc.sync.dma_start(a_sb[:], a_neg.ap())
            d_in_sb = cpool.tile([128, DT2], f32)
            nc.sync.dma_start(d_in_sb[:], d_in.ap())
            eye_sb = cpool.tile([128, 128], f32)
            nc.sync.dma_start(eye_sb[:], eye_d.ap())
            eye16 = cpool.tile([128, 128], bf16)
            nc.scalar.copy(eye16[:], eye_sb[:])
            sel_sb = cpool.tile([2 * NSTATE, 2 * NSTATE * 128], bf16)
            nc.sync.dma_start(sel_sb[:], sel_d.ap())
            # scan state carried across chunks: one column per (dtile, n)
            carry = cpool.tile([128, DT2 * NSTATE], f32)
            halo = cpool.tile([128, DT2 * (DCONV - 1)], f32)

            uT_ap = uT.ap().rearrange("(j p) t -> p j t", p=128)

            for b in range(BATCH):
                nc.vector.memset(carry[:], 0.0)
                for c in range(units_per_b):
                    t0 = b * seq_len + c * lc
                    # ---- load u chunk (all 1024 model dims, lc tokens) ----
                    u_sb = upool.tile([128, 8, lc], bf16, tag="u")
                    nc.sync.dma_start(u_sb[:], uT_ap[:, :, t0:t0 + lc])

                    # ---- in_proj: xz = W_in_shard @ u ----
                    x_sb = wpool.tile([128, DT2, lc + DCONV - 1], f32, tag="x")
                    z_sil = wpool.tile([128, DT2, lc], bf16, tag="z")
                    for mt in range(4):
                        ps = psmm.tile([128, lc], f32, tag="mm")
                        for j in range(8):
                            nc.tensor.matmul(
                                ps[:],
                                w_in_sb[:, j, 128 * mt:128 * (mt + 1)],
                                u_sb[:, j, :],
                                start=(j == 0), stop=(j == 7))
                        if mt < DT2:
                            nc.scalar.copy(x_sb[:, mt, DCONV - 1:], ps[:])
                        else:
                            nc.scalar.activation(z_sil[:, mt - DT2, :], ps[:],
                                                 AF.Silu, bias=0.0)

                    # ---- causal conv1d + silu ----
                    xs_sb = wpool.tile([128, DT2, lc], bf16, tag="xs")
                    for dt in range(DT2):
                        if c == 0:
                            nc.vector.memset(x_sb[:, dt, 0:DCONV - 1], 0.0)
                        else:
                            nc.vector.tensor_copy(
                                x_sb[:, dt, 0:DCONV - 1],
                                halo[:, dt * (DCONV - 1):(dt + 1) * (DCONV - 1)])
                        ca = spool.tile([128, lc], f32, tag="ca")
                        cb = spool.tile([128, lc], f32, tag="cb")
                        nc.vector.tensor_scalar_mul(
                            ca[:], x_sb[:, dt, 0:lc],
                            conv_w_sb[:, dt * DCONV:dt * DCONV + 1])
                        src, dst = ca, cb
                        for k in range(1, DCONV):
                            nc.vector.scalar_tensor_tensor(
                                dst[:], x_sb[:, dt, k:k + lc],
                                conv_w_sb[:, dt * DCONV + k:dt * DCONV + k + 1],
                                src[:], op0=OP.mult, op1=OP.add)
                            src, dst = dst, src
                        # update halo for next chunk (last DCONV-1 raw x cols)
                        nc.vector.tensor_copy(
                            halo[:, dt * (DCONV - 1):(dt + 1) * (DCONV - 1)],
                            x_sb[:, dt, lc:lc + DCONV - 1])
                        nc.scalar.activation(xs_sb[:, dt, :], src[:], AF.Silu,
                                             bias=conv_b_sb[:, dt:dt + 1])

                    # ---- x_proj partial + AllReduce ----
                    ps_xd = psx.tile([NXD, lc], f32, tag="xd")
                    for dt in range(DT2):
                        nc.tensor.matmul(ps_xd[:], w_xp_sb[:, dt, :],
                                         xs_sb[:, dt, :],
                                         start=(dt == 0), stop=(dt == DT2 - 1))
                    xd_loc = spool.tile([NXD, lc], bf16, tag="xdloc")
                    nc.scalar.copy(xd_loc[:], ps_xd[:])
                    ar_in = dpool.tile([NXD, lc], bf16, tag="arin")
                    ar_out = dpool.tile([NXD, lc], bf16, tag="arout")
                    nc.sync.dma_start(ar_in[:], xd_loc[:])
                    if fake_collective:
                        nc.sync.dma_start(ar_out[:], ar_in[:])
                    else:
                        nc.gpsimd.collective_compute(
                            "AllReduce", OP.add,
                            replica_groups=[list(range(NCORES))],
                            ins=[ar_in.opt()], outs=[ar_out.opt()])
                    xd_sb = spool.tile([DT_RANK, lc], bf16, tag="xdsb")
                    nc.sync.dma_start(xd_sb[:], ar_out[0:DT_RANK, :])
                    bc32 = spool.tile([2 * NSTATE, lc], bf16, tag="bc32")
                    nc.sync.dma_start(bc32[:],
                                      ar_out[DT_RANK:DT_RANK + 2 * NSTATE, :])

                    # ---- dt = softplus(dt_proj @ x_dbl[:, :64] + bias) ----
                    # softplus(v) == ln(1 + exp(v)); Exp and Ln share one
                    # ACT table (no Softplus table exists on gen3)
                    dt_sb = wpool.tile([128, DT2, lc], bf16, tag="dt")
                    for dt in range(DT2):
                        ps = psmm.tile([128, lc], f32, tag="mm")
                        nc.tensor.matmul(ps[:],
                                         w_dt_sb[:, 128 * dt:128 * (dt + 1)],
                                         xd_sb[0:DT_RANK, :],
                                         start=True, stop=True)
                        e_t = spool.tile([128, lc], f32, tag="esp")
                        nc.scalar.activation(e_t[:], ps[:], AF.Exp,
                                             bias=dt_bias_sb[:, dt:dt + 1])
                        nc.scalar.activation(dt_sb[:, dt, :], e_t[:], AF.Ln,
                                             bias=1.0)

                    dtx_sb = wpool.tile([128, DT2, lc], bf16, tag="dtx")
                    for dt in range(DT2):
                        nc.vector.tensor_mul(dtx_sb[:, dt, :], dt_sb[:, dt, :],
                                             xs_sb[:, dt, :])


                    # ---- selective scan over the chunk, per state n ----
                    y_ps = [psy.tile([128, lc], f32, tag="y", name=f"y_ps{i}")
                            for i in range(DT2)]
                    for n in range(NSTATE):
                        # broadcast B_n / C_n to 128 partitions via a k=32
                        # selector matmul on bc32, then one ACT cast to bf16
                        bc_ps = psb.tile([128, 2, lc], f32, tag="bcps")
                        nc.tensor.matmul(bc_ps[:, 0, :],
                                         sel_sb[:, 128 * n:128 * (n + 1)],
                                         bc32[:], start=True, stop=True)
                        nc.tensor.matmul(
                            bc_ps[:, 1, :],
                            sel_sb[:, 128 * (NSTATE + n):128 * (NSTATE + n + 1)],
                            bc32[:], start=True, stop=True)
                        bc2 = npool.tile([128, 2, lc], bf16, tag="bc2")
                        nc.scalar.copy(bc2[:], bc_ps[:])
                        bb = bc2[:, 0, :]
                        cc = bc2[:, 1, :]
                        for dt in range(DT2):
                            col = dt * NSTATE + n
                            a_t = npool.tile([128, lc], bf16, tag="a")
                            nc.scalar.activation(a_t[:], dt_sb[:, dt, :], AF.Exp,
                                                 bias=0.0,
                                                 scale=a_sb[:, col:col + 1])
                            dbx = npool.tile([128, lc], bf16, tag="dbx")
                            dbx_eng = nc.vector if (n % 2 == 0) else nc.gpsimd
                            dbx_eng.tensor_mul(dbx[:], dtx_sb[:, dt, :], bb[:])
                            h_t = npool.tile([128, lc], bf16, tag="h")
                            nc.vector.tensor_tensor_scan(
                                h_t[:], a_t[:], dbx[:],
                                initial=carry[:, col:col + 1],
                                op0=OP.mult, op1=OP.add)
                            nc.scalar.copy(carry[:, col:col + 1],
                                           h_t[:, lc - 1:lc])
                            w_t = npool.tile([128, lc], bf16, tag="w")
                            nc.gpsimd.tensor_mul(w_t[:], h_t[:], cc[:])
                            nc.tensor.matmul(y_ps[dt][:], eye16[:], w_t[:],
                                             start=(n == 0),
                                             stop=(n == NSTATE - 1))

                    # ---- y = y_ssm + D*xs, gate with silu(z) ----
                    yg = wpool.tile([128, DT2, lc], bf16, tag="yg")
                    for dt in range(DT2):
                        y16 = spool.tile([128, lc], bf16, tag="y16")
                        nc.scalar.copy(y16[:], y_ps[dt][:])
                        ys = spool.tile([128, lc], bf16, tag="ys")
                        nc.vector.scalar_tensor_tensor(
                            ys[:], xs_sb[:, dt, :], d_in_sb[:, dt:dt + 1],
                            y16[:], op0=OP.mult, op1=OP.add)
                        nc.gpsimd.tensor_mul(yg[:, dt, :], ys[:], z_sil[:, dt, :])

                    # ---- out_proj partial -> DRAM (host sums across cores) ----
                    for mt in range(8):
                        ps = psmm.tile([128, lc], f32, tag="mm")
                        for kt in range(DT2):
                            nc.tensor.matmul(
                                ps[:],
                                w_out_sb[:, kt, 128 * mt:128 * (mt + 1)],
                                yg[:, kt, :],
                                start=(kt == 0), stop=(kt == DT2 - 1))
                        ob = opool.tile([128, lc], bf16, tag="ob")
                        nc.scalar.copy(ob[:], ps[:])
                        nc.sync.dma_start(
                            y_part[128 * mt:128 * (mt + 1), t0:t0 + lc], ob[:])

    nc.compile()
    return nc


_CACHED = {}


def _get_nc(seq_len=SEQ, lc=512):
    key = (seq_len, lc)
    if key not in _CACHED:
        _CACHED[key] = _build_nc(seq_len, lc)
    return _CACHED[key]


def _host_prep(inputs, seq_len=SEQ):
    """Slice/transpose the full inputs into per-core in_maps."""
    import ml_dtypes
    _bf = ml_dtypes.bfloat16
    f32 = np.float32
    u = np.asarray(inputs["u"], f32)
    in_proj_w = np.asarray(inputs["in_proj_w"], f32)
    conv_w = np.asarray(inputs["conv_w"], f32)
    conv_b = np.asarray(inputs["conv_b"], f32)
    x_proj_w = np.asarray(inputs["x_proj_w"], f32)
    dt_proj_w = np.asarray(inputs["dt_proj_w"], f32)
    dt_bias = np.asarray(inputs["dt_bias"], f32)
    A_log = np.asarray(inputs["A_log"], f32)
    D_in = np.asarray(inputs["D_in"], f32)
    out_proj_w = np.asarray(inputs["out_proj_w"], f32)

    tok = BATCH * seq_len
    uT = np.ascontiguousarray(u.reshape(tok, D_MODEL).T).astype(_bf)
    eye = np.eye(128, dtype=f32)
    sel = np.kron(np.eye(2 * NSTATE, dtype=f32), np.ones((1, 128), f32)).astype(_bf)
    A = -np.exp(A_log)

    def fold(v):  # (256, k) -> (128, 2*k) with dtile-major columns
        v = v.reshape(DS, -1)
        return np.ascontiguousarray(
            np.concatenate([v[:128], v[128:]], axis=1))

    in_maps = []
    for k in range(NCORES):
        sl = slice(DS * k, DS * (k + 1))
        w_in_k = np.concatenate(
            [in_proj_w[sl], in_proj_w[D_INNER + DS * k:D_INNER + DS * (k + 1)]])
        in_maps.append({
            "uT": uT,
            "w_inT": np.ascontiguousarray(w_in_k.T).astype(_bf),
            "conv_w": fold(conv_w[sl]),
            "conv_b": fold(conv_b[sl]),
            "w_xpT": np.ascontiguousarray(x_proj_w[:, sl].T).astype(_bf),
            "w_dtT": np.ascontiguousarray(dt_proj_w[sl].T).astype(_bf),
            "dt_bias": fold(dt_bias[sl]),
            "a_neg": fold(A[sl]),
            "d_in": fold(D_in[sl]),
            "w_outT": np.ascontiguousarray(out_proj_w[:, sl].T).astype(_bf),
            "eye128": eye,
            "sel32": sel,
        })
    return in_maps


LAST_RESULTS = None


def bench(inputs, iters=24, warmup=4):
    """Estimate per-execution device time: device-put the sharded inputs
    once, then dispatch the jitted NEFF repeatedly (async) and time."""
    import time
    import jax
    import jax.numpy as jnp
    from jax.sharding import Mesh, PartitionSpec
    from jax.experimental.shard_map import shard_map
    import concourse.mybir as mybir
    from concourse import bass2jax
    from concourse.bass2jax import _bass_exec_p, install_neuronx_cc_hook

    install_neuronx_cc_hook()
    nc = _get_nc()
    in_maps = _host_prep(inputs)

    partition_name = (nc.partition_id_tensor.name
                      if nc.partition_id_tensor else None)
    in_names, out_names, out_avals, zero_outs = [], [], [], []
    for alloc in nc.m.functions[0].allocations:
        if not isinstance(alloc, mybir.MemoryLocationSet):
            continue
        name = alloc.memorylocations[0].name
        if alloc.kind == "ExternalInput":
            if name != partition_name:
                in_names.append(name)
        elif alloc.kind == "ExternalOutput":
            shape = tuple(alloc.tensor_shape)
            dtype = mybir.dt.np(alloc.dtype)
            out_avals.append(jax.core.ShapedArray(shape, dtype))
            out_names.append(name)
            zero_outs.append(np.zeros(shape, dtype))
    n_params = len(in_names)
    all_in_names = list(in_names) + list(out_names)
    if partition_name is not None:
        all_in_names.append(partition_name)

    def _body(*args):
        operands = list(args)
        if partition_name is not None:
            operands.append(bass2jax.partition_id_tensor())
        outs = _bass_exec_p.bind(
            *operands,
            out_avals=tuple(out_avals),
            in_names=tuple(all_in_names),
            out_names=tuple(out_names),
            lowering_input_output_aliases=(),
            sim_require_finite=True,
            sim_require_nnan=True,
            nc=nc,
        )
        return tuple(outs)

    devices = jax.devices()[:NCORES]
    mesh = Mesh(np.asarray(devices), ("core",))
    in_specs = (PartitionSpec("core"),) * (n_params + len(out_names))
    out_specs = (PartitionSpec("core"),) * len(out_names)
    fn = jax.jit(shard_map(_body, mesh=mesh, in_specs=in_specs,
                           out_specs=out_specs, check_rep=False),
                 keep_unused=True)

    concat_in = [np.concatenate([in_maps[c][nm] for c in range(NCORES)],
                                axis=0) for nm in in_names]
    concat_zeros = [np.zeros((NCORES * z.shape[0], *z.shape[1:]), z.dtype)
                    for z in zero_outs]
    from jax.sharding import NamedSharding
    sh = NamedSharding(mesh, PartitionSpec("core"))
    dev_in = [jax.device_put(a, sh) for a in concat_in + concat_zeros]

    for _ in range(warmup):
        outs = fn(*dev_in)
    jax.block_until_ready(outs)
    # two-point marginal: strips the large fixed per-batch dispatch
    # overhead of the axon proxy from the per-execution estimate
    times = {}
    for it in (iters // 4, iters):
        t0 = time.perf_counter()
        for _ in range(it):
            outs = fn(*dev_in)
        jax.block_until_ready(outs)
        times[it] = time.perf_counter() - t0
    ks = sorted(times)
    return (times[ks[1]] - times[ks[0]]) / (ks[1] - ks[0])


def kernel(**inputs):
    global LAST_RESULTS
    from concourse import bass_utils

    u = np.asarray(inputs["u"], np.float32)
    D_skip = np.asarray(inputs["D_skip"], np.float32)

    nc = _get_nc()
    in_maps = _host_prep(inputs)
    trace = bool(int(os.environ.get("MAMBA_TRACE", "0")))
    res = bass_utils.run_bass_kernel_spmd(
        nc, in_maps, core_ids=list(range(NCORES)), trace=trace)
    LAST_RESULTS = res

    acc = np.zeros((D_MODEL, BATCH * SEQ), np.float32)
    for r in res.results:
        acc += np.asarray(r["y_part"]).astype(np.float32)
    y = acc.T.reshape(BATCH, SEQ, D_MODEL)
    return y + D_skip[None, None, :] * u

